# revision 15
# baseline (speedup 1.0000x reference)
"""TRN2 Bass kernel for CausalSelfAttentionARMA.

Sharding: B(2) x head-groups(4 of 4 heads) -> 8 cores. Each core computes, for
its batch b and its 4 heads: q/k/k2 projections, causal softmax attention (AR),
the softmax-free linear-attention branch (MA) via a chunked recurrence, and a
partial output projection over its 256 head-channels. Host sums the 4 partials
per batch.

Math notes (vs reference):
 - softmax computed without max-subtraction (scores*scale are O(1), exp safe);
   masked entries are exactly 0 (affine_select after exp).
 - MA branch: w = tril(qa ka^T), y2 = w e computed as chunked linear attention
   with a [64,64] running state S = sum ka e^T, instead of the TxT matmul.
 - qa' := leaky_relu(-q*scale) = -qa is used everywhere; signs cancel in pairs
   (the y2 psum holds -y2^T and the final merge subtracts it).
 - final projection computed once on (y + y2).
All matmuls bf16 (fp32 psum accumulation); validated end-to-end rel err ~5e-3
of absmax vs the fp32 reference.
"""
import os
import sys
from contextlib import ExitStack

import numpy as np

sys.path.insert(0, "/opt/trn_rl_repo")

import concourse.bass as bass
from concourse import bacc
import concourse.mybir as mybir
import concourse.tile as tile

BF = mybir.dt.bfloat16
F32 = mybir.dt.float32
AF = mybir.ActivationFunctionType
ALU = mybir.AluOpType

T = 2048
C = 1024
D = 64
NH = 4          # heads per core
TI = T // 128   # 16 partition tiles
TB = T // 512   # 4 query chunks
SCALE = 0.125   # 1/sqrt(64)
MA_K = 0.02
SLOPE = 0.02
N_CORES = 8


def build_kernel(nc: bass.Bass, tc: tile.TileContext, ctx: ExitStack,
                 x_b, xv, wqk, wk2, wproj, outp):
    sb = ctx.enter_context(tc.tile_pool(name="sb", bufs=1))
    stage = ctx.enter_context(tc.tile_pool(name="stage", bufs=3))
    pt_pool = ctx.enter_context(tc.tile_pool(name="pt", bufs=6))
    outs_pool = ctx.enter_context(tc.tile_pool(name="outs", bufs=3))

    # ---------------- phase 0: loads, casts, transposes ----------------
    xn = sb.tile([128, TI, C], BF)
    for ti in range(TI):
        nc.gpsimd.dma_start(out=xn[:, ti, :], in_=x_b[ti * 128:(ti + 1) * 128, :])
    xT = sb.tile([128, 8, T], BF)
    for ti in range(TI):
        nc.sync.dma_start_transpose(out=xT[:, :, ti * 128:(ti + 1) * 128],
                                    in_=xn[:, ti, :])
    vnat = sb.tile([128, TI, NH * D], BF)
    nc.gpsimd.dma_start(out=vnat[:],
                        in_=xv[0:T, :].rearrange("(ti p) c -> p ti c", p=128))
    wqk_s = sb.tile([128, 8, 512], BF)
    nc.gpsimd.dma_start(out=wqk_s[:], in_=wqk.rearrange("(ci p) n -> p ci n", p=128))
    wk2_s = sb.tile([128, 8, 256], BF)
    nc.gpsimd.dma_start(out=wk2_s[:], in_=wk2.rearrange("(ci p) n -> p ci n", p=128))
    wproj_s = sb.tile([128, 2, C], BF)
    nc.gpsimd.dma_start(out=wproj_s[:], in_=wproj.rearrange("(ci p) n -> p ci n", p=128))

    # v_ext: [128, ti, h, 65] = [v | ones] stationary for the PV matmul
    v_ext = sb.tile([128, TI, NH, D + 1], BF)
    nc.vector.memset(v_ext[:, :, :, D:D + 1], 1.0)
    for ti in range(TI):
        nc.vector.tensor_copy(
            v_ext[:, ti, :, 0:D],
            vnat[:, ti, :].rearrange("p (h d) -> p h d", d=D))
    ones1 = sb.tile([1, D], BF)
    nc.vector.memset(ones1[:], 1.0)

    # ---------------- phase A: projections ----------------
    qT = sb.tile([128, 2, T], BF)
    kT = sb.tile([128, 2, T], BF)
    qaT = sb.tile([128, 2, T], BF)
    kaT = sb.tile([128, 2, T], BF)

    with tc.tile_pool(name="pa_ps", bufs=3, space="PSUM") as pa_ps:
        for si in range(2):          # head-pair stacks
            for tc4 in range(TB):    # 512-wide t chunks
                tsl = slice(tc4 * 512, (tc4 + 1) * 512)
                # q stack
                ps = pa_ps.tile([128, 512], F32)
                for ci in range(8):
                    nc.tensor.matmul(ps[:], lhsT=wqk_s[:, ci, si * 128:(si + 1) * 128],
                                     rhs=xT[:, ci, tsl], start=(ci == 0), stop=(ci == 7))
                nc.vector.tensor_copy(qT[:, si, tsl], ps[:])
                r = stage.tile([128, 512], BF, tag="lrelu")
                nc.scalar.activation(r[:], ps[:], AF.Relu, scale=-SCALE * (1.0 - SLOPE))
                nc.vector.scalar_tensor_tensor(out=qaT[:, si, tsl], in0=ps[:],
                                               scalar=-SCALE * SLOPE, in1=r[:],
                                               op0=ALU.mult, op1=ALU.add)
                # k stack
                ps = pa_ps.tile([128, 512], F32)
                for ci in range(8):
                    nc.tensor.matmul(ps[:], lhsT=wqk_s[:, ci, 256 + si * 128:256 + (si + 1) * 128],
                                     rhs=xT[:, ci, tsl], start=(ci == 0), stop=(ci == 7))
                nc.vector.tensor_copy(kT[:, si, tsl], ps[:])
                # k2 stack -> ka = sigmoid(MA_K*scale*k2)
                ps = pa_ps.tile([128, 512], F32)
                for ci in range(8):
                    nc.tensor.matmul(ps[:], lhsT=wk2_s[:, ci, si * 128:(si + 1) * 128],
                                     rhs=xT[:, ci, tsl], start=(ci == 0), stop=(ci == 7))
                nc.scalar.activation(kaT[:, si, tsl], ps[:], AF.Sigmoid, scale=MA_K * SCALE)

    ka_nat = sb.tile([128, 2, TI, 128], BF)
    for si in range(2):
        nc.sync.dma_start_transpose(out=ka_nat[:, si, :, :], in_=kaT[:, si, :])

    # ---------------- phase B: AR branch (flash-style causal softmax) ----------
    yTn = sb.tile([128, 2, T], BF)
    with tc.tile_pool(name="st_ps", bufs=3, space="PSUM") as st_ps, \
         tc.tile_pool(name="ye_ps", bufs=2, space="PSUM") as ye_ps, \
         tc.tile_pool(name="rz_ps", bufs=2, space="PSUM") as rz_ps:
        for h in range(NH):
            si, r0 = h // 2, (h % 2) * 64
            for tb in range(TB):
                tsl = slice(tb * 512, (tb + 1) * 512)
                nsb = 4 * (tb + 1)
                yext = ye_ps.tile([65, 512], F32)
                for sbk in range(nsb):
                    stp = st_ps.tile([128, 512], F32)
                    nc.tensor.matmul(stp[:],
                                     lhsT=kT[r0:r0 + 64, si, sbk * 128:(sbk + 1) * 128],
                                     rhs=qT[r0:r0 + 64, si, tsl], start=True, stop=True)
                    pt = pt_pool.tile([128, 512], BF)
                    nc.scalar.activation(pt[:], stp[:], AF.Exp, scale=SCALE)
                    if sbk >= 4 * tb:  # diagonal band: zero where t < s
                        nc.gpsimd.affine_select(
                            out=pt[:], in_=pt[:], compare_op=ALU.is_ge, fill=0.0,
                            base=tb * 512 - sbk * 128, channel_multiplier=-1,
                            pattern=[[1, 512]])
                    nc.tensor.matmul(yext[:], lhsT=v_ext[:, sbk, h, :], rhs=pt[:],
                                     start=(sbk == 0), stop=(sbk == nsb - 1))
                rz = stage.tile([1, 512], BF, tag="rz")
                with nc.allow_low_precision(reason="softmax denom to bf16"):
                    nc.vector.reciprocal(rz[:], yext[64:65, :])
                rzb = rz_ps.tile([64, 512], F32)
                nc.tensor.matmul(rzb[:], lhsT=ones1[:], rhs=rz[:], start=True, stop=True)
                rzs = stage.tile([64, 512], BF, tag="rzs")
                nc.vector.tensor_copy(rzs[:], rzb[:])
                nc.vector.tensor_tensor(out=yTn[r0:r0 + 64, si, tsl],
                                        in0=yext[0:64, :], in1=rzs[:], op=ALU.mult)

    y_nat = sb.tile([128, 2, TI, 128], BF)
    for si in range(2):
        nc.sync.dma_start_transpose(out=y_nat[:, si, :, :], in_=yTn[:, si, :])

    # ---------------- e = v[1:] - y[:-1] (shifted) ----------------
    vs = sb.tile([128, TI, NH * D], BF)
    nc.gpsimd.dma_start(out=vs[:],
                        in_=xv[1:T + 1, :].rearrange("(ti p) c -> p ti c", p=128))
    e_t = sb.tile([128, TI, NH, D], BF)
    for h in range(NH):
        si, hc = h // 2, (h % 2) * 64
        nc.vector.tensor_tensor(out=e_t[:, :, h, :],
                                in0=vs[:, :, h * D:(h + 1) * D],
                                in1=y_nat[:, si, :, hc:hc + 64], op=ALU.subtract)

    # ---------------- phase C: MA branch, chunked linear recurrence --------
    mergedT = sb.tile([128, 2, T], BF)
    nc.vector.tensor_copy(mergedT[:, :, 0:1], yTn[:, :, 0:1])
    s_bf = sb.tile([128, 2, 64], BF)
    s_f32 = sb.tile([64, NH, 64], F32)
    nc.vector.memset(s_f32[:], 0.0)
    with tc.tile_pool(name="s_ps", bufs=2, space="PSUM") as s_ps, \
         tc.tile_pool(name="gt_ps", bufs=2, space="PSUM") as gt_ps, \
         tc.tile_pool(name="y2_ps", bufs=2, space="PSUM") as y2_ps:
        for ci in range(TI):
            c0 = 128 * ci
            sz = 127 if ci == TI - 1 else 128
            for h in range(NH):
                si, r0, hc = h // 2, (h % 2) * 64, (h % 2) * 64
                qa_sl = qaT[r0:r0 + 64, si, 1 + c0:1 + c0 + sz]
                ka_sl = kaT[r0:r0 + 64, si, c0:c0 + sz]
                kan_sl = ka_nat[0:sz, si, ci, hc:hc + 64]
                e_sl = e_t[0:sz, ci, h, :]
                y2p = y2_ps.tile([64, 128], F32)
                if ci > 0:
                    nc.tensor.matmul(y2p[:, 0:sz], lhsT=s_bf[r0:r0 + 64, si, :],
                                     rhs=qa_sl, start=True, stop=False)
                gt = gt_ps.tile([128, 128], F32)
                nc.tensor.matmul(gt[0:sz, 0:sz], lhsT=ka_sl, rhs=qa_sl,
                                 start=True, stop=True)
                gts = stage.tile([128, 128], BF, tag="gts")
                nc.scalar.activation(gts[0:sz, 0:sz], gt[0:sz, 0:sz], AF.Copy)
                nc.gpsimd.affine_select(out=gts[0:sz, 0:sz], in_=gts[0:sz, 0:sz],
                                        compare_op=ALU.is_ge, fill=0.0, base=0,
                                        channel_multiplier=-1, pattern=[[1, sz]])
                nc.tensor.matmul(y2p[:, 0:sz], lhsT=e_sl, rhs=gts[0:sz, 0:sz],
                                 start=(ci == 0), stop=True)
                # state update S += ka_chunk^T e_chunk, then snapshot to bf16
                if ci < TI - 1:
                    ds = s_ps.tile([64, 64], F32)
                    nc.tensor.matmul(ds[:], lhsT=kan_sl, rhs=e_sl,
                                     start=True, stop=True)
                    nc.vector.tensor_tensor(out=s_f32[:, h, :], in0=s_f32[:, h, :],
                                            in1=ds[:], op=ALU.add)
                    nc.vector.tensor_copy(s_bf[r0:r0 + 64, si, :], s_f32[:, h, :])
                # merge: mergedT = yTn - (-y2T)
                nc.vector.tensor_tensor(
                    out=mergedT[r0:r0 + 64, si, 1 + c0:1 + c0 + sz],
                    in0=yTn[r0:r0 + 64, si, 1 + c0:1 + c0 + sz],
                    in1=y2p[:, 0:sz], op=ALU.subtract)

    # ---------------- output projection (partial over this core's channels) ----
    with tc.tile_pool(name="pj_ps", bufs=4, space="PSUM") as pj_ps:
        for t16 in range(TI):
            for n2 in range(2):
                pp = pj_ps.tile([128, 512], F32)
                for chi in range(2):
                    nc.tensor.matmul(pp[:],
                                     lhsT=mergedT[:, chi, t16 * 128:(t16 + 1) * 128],
                                     rhs=wproj_s[:, chi, n2 * 512:(n2 + 1) * 512],
                                     start=(chi == 0), stop=(chi == 1))
                ot = outs_pool.tile([128, 512], F32)
                nc.vector.tensor_copy(ot[:], pp[:])
                nc.sync.dma_start(
                    out=outp[t16 * 128:(t16 + 1) * 128, n2 * 512:(n2 + 1) * 512],
                    in_=ot[:])


_CACHE = {}


def build_program():
    if "nc" in _CACHE:
        return _CACHE["nc"]
    nc = bacc.Bacc("TRN2", target_bir_lowering=False, debug=False)
    x_b = nc.dram_tensor("x_b", [T, C], F32, kind="ExternalInput").ap()
    xv = nc.dram_tensor("xv", [T + 128, NH * D], F32, kind="ExternalInput").ap()
    wqk = nc.dram_tensor("wqk", [C, 512], F32, kind="ExternalInput").ap()
    wk2 = nc.dram_tensor("wk2", [C, 256], F32, kind="ExternalInput").ap()
    wproj = nc.dram_tensor("wproj", [NH * D, C], F32, kind="ExternalInput").ap()
    outp = nc.dram_tensor("outp", [T, C], F32, kind="ExternalOutput").ap()
    with tile.TileContext(nc) as tc, ExitStack() as ctx:
        build_kernel(nc, tc, ctx, x_b, xv, wqk, wk2, wproj, outp)
    nc.compile()
    _CACHE["nc"] = nc
    return nc


def make_in_maps(x, W_attn, W_k2, W_proj):
    x = np.ascontiguousarray(np.asarray(x, dtype=np.float32))
    W_attn = np.asarray(W_attn, dtype=np.float32)
    W_k2 = np.asarray(W_k2, dtype=np.float32)
    W_proj = np.asarray(W_proj, dtype=np.float32)
    in_maps = []
    for core in range(N_CORES):
        b, g = core // 4, core % 4
        h0 = g * NH * D
        wqk = np.ascontiguousarray(
            np.concatenate([W_attn[:, h0:h0 + 256], W_attn[:, C + h0:C + h0 + 256]],
                           axis=1))
        in_maps.append({
            "x_b": np.ascontiguousarray(x[b]),
            "xv": np.ascontiguousarray(
                np.pad(x[b][:, h0:h0 + 256], ((0, 128), (0, 0)))),
            "wqk": wqk,
            "wk2": np.ascontiguousarray(W_k2[:, h0:h0 + 256]),
            "wproj": np.ascontiguousarray(W_proj[h0:h0 + 256, :]),
        })
    return in_maps


def run_sharded(x, W_attn, W_k2, W_proj, trace=False):
    from concourse.bass_utils import run_bass_kernel_spmd
    nc = build_program()
    in_maps = make_in_maps(x, W_attn, W_k2, W_proj)
    res = run_bass_kernel_spmd(nc, in_maps, list(range(N_CORES)), trace=trace)
    out = np.zeros((2, T, C), np.float32)
    for core in range(N_CORES):
        out[core // 4] += res.results[core]["outp"]
    return out, res


def kernel(x, W_attn, W_k2, W_proj):
    out, _ = run_sharded(x, W_attn, W_k2, W_proj, trace=False)
    return out
